# revision 1
# baseline (speedup 1.0000x reference)
"""ColumnParallelFusedMoeLinear grouped-GEMM kernel for 8 Trainium2 NeuronCores.

Strategy (expert/token parallel, not column parallel):
  Tokens are sorted by expert; m_sizes gives each expert's contiguous row
  range of x.  The host splits the full token range into 8 contiguous chunks,
  each served by one core, allocated proportionally to per-expert segment
  sizes so every chunk maps to exactly ONE expert (balanced m_sizes -> one
  expert per core; a skewed expert gets split across several cores along
  tokens).  Each core computes y_chunk = x_chunk @ weight[e].T as a dense
  matmul and the host scatters chunk rows back into the full output.

  Per-core HBM traffic = x_chunk (~4.5 MB) + one weight (8 MB) + y_chunk
  (~9 MB), which is the global minimum (x, weight, y each touched once
  across the chip).

  The matmuls run in float32r (fp32 data, PE rounds to 11-bit mantissa,
  full 1 column/cycle rate at free dim >= 256) with fp32 PSUM accumulation.

  Host pre-transposes x-chunks and weights so the contraction dim (D_IN)
  is the partition dim for both operands and every DMA is wide/contiguous.
"""

import math

import numpy as np

_N_CORES = 8
_P = 128
_NFREE = 512  # matmul moving free dim / PSUM bank width in fp32

# (M_pad, D_IN, D_OUT) -> (nc, in_names) compiled-program cache so repeated
# kernel() calls in one process reuse the traced module (and hence the
# process-level NEFF compile cache).
_program_cache = {}


def _build_program(m_pad, d_in, d_out, out_engine="gpsimd", w_merge_tail=False,
                   out_pair=False):
    import concourse.mybir as mybir
    import concourse.tile as tile
    from concourse import bacc

    kc_n = d_in // _P          # contraction chunks of 128
    mt_n = m_pad // _P         # token tiles
    nt_n = d_out // _NFREE     # output-column tiles

    nc = bacc.Bacc("TRN2", target_bir_lowering=False, debug=False)
    xT = nc.dram_tensor("xT", [d_in, m_pad], mybir.dt.float32r, kind="ExternalInput")
    wT = nc.dram_tensor("wT", [d_in, d_out], mybir.dt.float32r, kind="ExternalInput")
    y = nc.dram_tensor("y", [m_pad, d_out], mybir.dt.float32, kind="ExternalOutput")

    xT3 = xT.rearrange("(kc p) m -> kc p m", p=_P)
    wT3 = wT.rearrange("(kc p) o -> kc p o", p=_P)
    y3 = y.rearrange("(mt p) o -> mt p o", p=_P)

    # x columns are DMA'd in two groups per k-chunk; the compute loop runs the
    # head m-tiles through ALL weight columns first, then the tail m-tiles.
    # This spreads both the x and w input streams across the whole timeline
    # (peak early-bandwidth demand is what stalls the PE otherwise).
    XG_HEAD = min((mt_n + 1) // 2 + 1, mt_n)

    with tile.TileContext(nc) as tc:
        with (
            tc.tile_pool(name="xw", bufs=1) as xwpool,
            tc.tile_pool(name="out", bufs=8) as outpool,
            tc.tile_pool(name="psum", bufs=8, space="PSUM") as psumpool,
        ):
            wt = {}
            xh = [[None] * kc_n, [None] * kc_n]  # head / tail x tiles per kc

            def load_w(nts, kc):
                """One DMA covering weight columns nts (a contiguous list)."""
                n0, n1 = nts[0], nts[-1] + 1
                t = xwpool.tile([_P, (n1 - n0) * _NFREE], mybir.dt.float32r,
                                tag=f"w{kc}_{n0}")
                bi = nc.sync.dma_start(t[:], wT3[kc, :, n0 * _NFREE:n1 * _NFREE])
                in_dma_insts.append(bi.ins)
                for nt in nts:
                    wt[(kc, nt)] = t[:, (nt - n0) * _NFREE:(nt - n0 + 1) * _NFREE]

            def load_xh(h, kc):
                c0 = 0 if h == 0 else XG_HEAD * _P
                c1 = XG_HEAD * _P if h == 0 else m_pad
                t = xwpool.tile([_P, c1 - c0], mybir.dt.float32r, tag=f"x{kc}_{h}")
                bi = nc.sync.dma_start(t[:], xT3[kc, :, c0:c1])
                in_dma_insts.append(bi.ins)
                xh[h][kc] = t

            in_dma_insts = []

            # DMA emission in PE need-order.  Transfers serialize at ~full
            # HBM bandwidth, so program order == arrival order.  The first
            # k-loop needs x(kc, m0..) + w(kc, n0) pairwise, so interleave
            # those; the x remainder and later weight columns follow in
            # consumption order (the last two columns merged into one DMA —
            # they arrive with plenty of slack).
            for kc in range(kc_n):
                load_xh(0, kc)
                load_w([0], kc)
            if nt_n > 1:
                for kc in range(kc_n):
                    load_w([1], kc)
            if nt_n > 2:
                if w_merge_tail:
                    for kc in range(kc_n):
                        load_w(list(range(2, nt_n)), kc)
                else:
                    for nt in range(2, nt_n):
                        for kc in range(kc_n):
                            load_w([nt], kc)
            if mt_n > XG_HEAD:
                for kc in range(kc_n):
                    load_xh(1, kc)

            out_dma = {
                "gpsimd": nc.gpsimd.dma_start,
                "scalar": nc.scalar.dma_start,
                "scalar_ordered": nc.scalar.dma_start,
                "sync": nc.sync.dma_start,
            }[out_engine]
            last_in_dma = in_dma_insts[-1]
            halves = [(0, XG_HEAD)]
            if XG_HEAD < mt_n:
                halves.append((XG_HEAD, mt_n))
            for h0, h1 in halves:
              for nt in range(nt_n):
                mt = h0
                while mt < h1:
                    # pair adjacent m-tiles into one store tile
                    npair = 2 if (out_pair and mt + 1 < h1) else 1
                    o = outpool.tile([_P, npair * _NFREE], mybir.dt.float32, tag="o")
                    for j in range(npair):
                        r_mt = mt + j
                        if r_mt < XG_HEAD:
                            lhs_tile, r = xh[0], r_mt
                        else:
                            lhs_tile, r = xh[1], r_mt - XG_HEAD
                        ps = psumpool.tile([_P, _NFREE], mybir.dt.float32, tag="ps")
                        for kc in range(kc_n):
                            nc.tensor.matmul(
                                ps[:],
                                lhs_tile[kc][:, r * _P:(r + 1) * _P],
                                wt[(kc, nt)],
                                start=(kc == 0),
                                stop=(kc == kc_n - 1),
                            )
                        nc.vector.tensor_copy(o[:, j * _NFREE:(j + 1) * _NFREE], ps[:])
                    dst = y3[mt:mt + npair, :, nt * _NFREE:(nt + 1) * _NFREE]
                    bi = out_dma(
                        dst.rearrange("t p n -> p t n"),
                        o[:].rearrange("p (t n) -> p t n", t=npair),
                    )
                    if out_engine == "scalar_ordered":
                        # schedule-order all stores after every input load so
                        # the HWDGE lane rotation never chains a load behind
                        # a store
                        tile.add_dep_helper(
                            bi.ins, last_in_dma, sync=False,
                            reason="stores after loads for clean DMA lanes",
                        )
                    mt += npair
    nc.compile()
    return nc


# Largest chunk one core handles per SPMD round; 8 k-chunks of x at this
# width (4 B elements) stay well inside the 192 KB/partition SBUF budget
# next to the resident weight tiles.
_MAX_CHUNK = 2560


def _plan_chunks(m_sizes, T):
    """Split [0, T) into single-expert chunks, balanced by length.

    Every chunk is <= _MAX_CHUNK rows.  Returns a list of (expert, row0,
    row1) padded with empty (0, 0, 0) chunks to a multiple of _N_CORES,
    or None if there are no rows at all.
    """
    off = np.cumsum(np.asarray(m_sizes, dtype=np.int64))
    starts = np.clip(np.concatenate([[0], off[:-1]]), 0, T)
    ends = np.clip(off, 0, T)
    segs = [(e, int(starts[e]), int(ends[e]))
            for e in range(len(m_sizes)) if ends[e] > starts[e]]
    if not segs:
        return None
    lens = np.array([s1 - s0 for _, s0, s1 in segs], dtype=np.float64)
    # mandatory splits so no chunk exceeds _MAX_CHUNK, then distribute any
    # spare cores (up to the next multiple of _N_CORES) to the biggest shares
    n_chunks = np.ceil(lens / _MAX_CHUNK).astype(np.int64)
    total = int(n_chunks.sum())
    spare = (-total) % _N_CORES if total > _N_CORES else _N_CORES - total
    for _ in range(spare):
        i = int(np.argmax(lens / n_chunks))
        n_chunks[i] += 1
    chunks = []
    for (e, s0, s1), k in zip(segs, n_chunks):
        L = s1 - s0
        bounds = [s0 + (L * i) // k for i in range(int(k) + 1)]
        for i in range(int(k)):
            if bounds[i + 1] > bounds[i]:
                chunks.append((e, bounds[i], bounds[i + 1]))
    while len(chunks) % _N_CORES:
        chunks.append((0, 0, 0))
    return chunks


def kernel(x, weight, m_sizes):
    from concourse.bass_utils import run_bass_kernel_spmd

    x = np.ascontiguousarray(np.asarray(x), dtype=np.float32)
    weight = np.ascontiguousarray(np.asarray(weight), dtype=np.float32)
    m_arr = np.asarray(m_sizes)

    T, d_in = x.shape
    E, d_out, _ = weight.shape

    y = np.zeros((T, d_out), dtype=np.float32)
    chunks = _plan_chunks(m_arr, T)
    if chunks is None:
        return y

    max_len = max(r1 - r0 for _, r0, r1 in chunks)
    m_pad = max(_P, int(math.ceil(max_len / _P)) * _P)

    import os
    out_engine = os.environ.get("MOE_OUT_ENGINE", "scalar_ordered")
    w_merge_tail = os.environ.get("MOE_W_MERGE", "1") == "1"
    out_pair = os.environ.get("MOE_OUT_PAIR", "1") == "1"
    key = (m_pad, d_in, d_out, out_engine, w_merge_tail, out_pair)
    if key not in _program_cache:
        _program_cache[key] = _build_program(
            m_pad, d_in, d_out, out_engine, w_merge_tail, out_pair
        )
    nc = _program_cache[key]

    # weight[e].T, C-contiguous, built once per expert actually used
    wT_cache = {}
    for round0 in range(0, len(chunks), _N_CORES):
        batch = chunks[round0:round0 + _N_CORES]
        in_maps = []
        for e, r0, r1 in batch:
            xT = np.zeros((d_in, m_pad), dtype=np.float32)
            if r1 > r0:
                xT[:, : r1 - r0] = x[r0:r1].T
            if e not in wT_cache:
                wT_cache[e] = np.ascontiguousarray(weight[e].T)
            in_maps.append({"xT": xT, "wT": wT_cache[e]})

        res = run_bass_kernel_spmd(nc, in_maps, core_ids=list(range(_N_CORES)))

        for (e, r0, r1), out in zip(batch, res.results):
            if r1 > r0:
                y[r0:r1] = out["y"][: r1 - r0]
    return y



# revision 5
# speedup vs baseline: 1.1141x; 1.1141x over previous
"""ColumnParallelFusedMoeLinear grouped-GEMM kernel for 8 Trainium2 NeuronCores.

Strategy (expert/token parallel):
  Tokens are sorted by expert; m_sizes gives each expert's contiguous row
  range of x.  The host splits the full token range into single-expert
  chunks (balanced m_sizes -> one expert per core) and each core computes
  y_chunk = x_chunk @ weight[e].T, with the host scattering chunk rows back
  into the full output.

  Inputs are cast to bf16 on the host (PE streams 1 column/cycle for bf16
  same as fp32r, but HBM traffic halves; K=1024 accumulation stays fp32 in
  PSUM so the result error is ~5e-3, well inside the 2e-2 gate).  The
  output is stored transposed (yT, bf16) and the host casts/transposes it
  back, halving the store traffic too.  Per-core HBM traffic ~10 MB vs a
  ~57 us matmul roofline, so the kernel is tensor-engine-bound.

  The matmul puts the weight column block [k=128, n=128] stationary and
  streams the token dim as the moving free dim, so the ragged per-expert
  token count (973..1065 here) is NOT padded up to a multiple of 128 --
  only to the SPMD-uniform m_pad.  PSUM limits the moving dim to 512 fp32
  per bank, so the token range is cut into ceil(m_pad/512) near-equal
  chunks (all >=256 wide, keeping LDWEIGHTS hidden under the stream).
"""

import math
import os

import ml_dtypes
import numpy as np

_N_CORES = 8
_P = 128
_PSUM_F = 512  # PSUM bank width in fp32 = max moving free dim per matmul

_program_cache = {}


def _m_chunks(m_pad):
    """Cut [0, m_pad) into near-equal chunks of <=512, multiples of 8."""
    n = (m_pad + _PSUM_F - 1) // _PSUM_F
    base = min(_PSUM_F, ((-(-m_pad // n) + 7) // 8) * 8)
    bounds = []
    c = 0
    while c < m_pad:
        bounds.append((c, min(c + base, m_pad)))
        c = bounds[-1][1]
    return bounds


def _build_program(m_pad, d_in, d_out, out_engine="gpsimd"):
    import concourse.mybir as mybir
    import concourse.tile as tile
    from concourse import bacc

    kc_n = d_in // _P   # contraction chunks of 128
    nt_n = d_out // _P  # stationary weight column tiles
    chunks = _m_chunks(m_pad)

    nc = bacc.Bacc("TRN2", target_bir_lowering=False, debug=False)
    xT = nc.dram_tensor("xT", [d_in, m_pad], mybir.dt.bfloat16, kind="ExternalInput")
    wT = nc.dram_tensor("wT", [d_in, d_out], mybir.dt.bfloat16, kind="ExternalInput")
    yT = nc.dram_tensor("yT", [d_out, m_pad], mybir.dt.bfloat16, kind="ExternalOutput")

    xT3 = xT.rearrange("(kc p) m -> kc p m", p=_P)
    wT3 = wT.rearrange("(kc p) o -> kc p o", p=_P)
    y3 = yT.rearrange("(nt p) m -> nt p m", p=_P)

    with tile.TileContext(nc) as tc:
        with (
            tc.tile_pool(name="xw", bufs=1) as xwpool,
            tc.tile_pool(name="out", bufs=4) as outpool,
            tc.tile_pool(name="psum", bufs=6, space="PSUM") as psumpool,
        ):
            xt = [xwpool.tile([_P, m_pad], mybir.dt.bfloat16, tag=f"x{kc}",
                              name=f"x{kc}") for kc in range(kc_n)]
            wt = [xwpool.tile([_P, d_out], mybir.dt.bfloat16, tag=f"w{kc}",
                              name=f"w{kc}") for kc in range(kc_n)]

            # DMA emission order == arrival order (transfers serialize near
            # full HBM bandwidth on the sync queue).  Ramp: the first weight
            # column tile and the first x chunk pairwise per kc, so the PE
            # can start after ~300 KB; then the x remainder (needed by the
            # 2nd psum group), then the remaining weight columns in
            # geometrically growing n-groups in consumption order.
            c0, c1 = chunks[0]
            for kc in range(kc_n):
                nc.sync.dma_start(wt[kc][:, 0:_P], wT3[kc, :, 0:_P])
                nc.sync.dma_start(xt[kc][:, c0:c1], xT3[kc, :, c0:c1])
            if c1 < m_pad:
                for kc in range(kc_n):
                    nc.sync.dma_start(xt[kc][:, c1:m_pad], xT3[kc, :, c1:m_pad])
            g0 = 1
            while g0 < nt_n:
                g1 = min(2 * g0, nt_n)
                for kc in range(kc_n):
                    nc.sync.dma_start(wt[kc][:, g0 * _P:g1 * _P],
                                      wT3[kc, :, g0 * _P:g1 * _P])
                g0 = g1

            out_dma = {
                "gpsimd": nc.gpsimd.dma_start,
                "scalar": nc.scalar.dma_start,
                "sync": nc.sync.dma_start,
            }[out_engine]

            for nt in range(nt_n):
                ps = [psumpool.tile([_P, _PSUM_F], mybir.dt.float32, tag="ps",
                                    name=f"ps{nt}_{ci}")
                      for ci in range(len(chunks))]
                o = outpool.tile([_P, m_pad], mybir.dt.bfloat16, tag="o")
                for ci, (c0, c1) in enumerate(chunks):
                    for kc in range(kc_n):
                        nc.tensor.matmul(
                            ps[ci][:, :c1 - c0],
                            wt[kc][:, nt * _P:(nt + 1) * _P],
                            xt[kc][:, c0:c1],
                            start=(kc == 0),
                            stop=(kc == kc_n - 1),
                        )
                    nc.vector.tensor_copy(o[:, c0:c1], ps[ci][:, :c1 - c0])
                    out_dma(y3[nt, :, c0:c1], o[:, c0:c1])
    nc.compile()
    return nc


# Largest chunk one core handles per SPMD round (SBUF-bounded).
_MAX_CHUNK = 2560


def _plan_chunks(m_sizes, T):
    """Split [0, T) into single-expert chunks, balanced by length.

    Every chunk is <= _MAX_CHUNK rows.  Returns a list of (expert, row0,
    row1) padded with empty (0, 0, 0) chunks to a multiple of _N_CORES,
    or None if there are no rows at all.
    """
    off = np.cumsum(np.asarray(m_sizes, dtype=np.int64))
    starts = np.clip(np.concatenate([[0], off[:-1]]), 0, T)
    ends = np.clip(off, 0, T)
    segs = [(e, int(starts[e]), int(ends[e]))
            for e in range(len(m_sizes)) if ends[e] > starts[e]]
    if not segs:
        return None
    lens = np.array([s1 - s0 for _, s0, s1 in segs], dtype=np.float64)
    # mandatory splits so no chunk exceeds _MAX_CHUNK, then distribute any
    # spare cores (up to the next multiple of _N_CORES) to the biggest shares
    n_chunks = np.ceil(lens / _MAX_CHUNK).astype(np.int64)
    total = int(n_chunks.sum())
    spare = (-total) % _N_CORES if total > _N_CORES else _N_CORES - total
    for _ in range(spare):
        i = int(np.argmax(lens / n_chunks))
        n_chunks[i] += 1
    chunks = []
    for (e, s0, s1), k in zip(segs, n_chunks):
        L = s1 - s0
        bounds = [s0 + (L * i) // k for i in range(int(k) + 1)]
        for i in range(int(k)):
            if bounds[i + 1] > bounds[i]:
                chunks.append((e, bounds[i], bounds[i + 1]))
    while len(chunks) % _N_CORES:
        chunks.append((0, 0, 0))
    return chunks


def kernel(x, weight, m_sizes):
    from concourse.bass_utils import run_bass_kernel_spmd

    x = np.ascontiguousarray(np.asarray(x), dtype=np.float32)
    weight = np.asarray(weight, dtype=np.float32)
    m_arr = np.asarray(m_sizes)

    T, d_in = x.shape
    E, d_out, _ = weight.shape

    y = np.zeros((T, d_out), dtype=np.float32)
    chunks = _plan_chunks(m_arr, T)
    if chunks is None:
        return y

    max_len = max(r1 - r0 for _, r0, r1 in chunks)
    m_pad = max(_P, int(math.ceil(max_len / 16)) * 16)

    out_engine = os.environ.get("MOE_OUT_ENGINE", "scalar")
    key = (m_pad, d_in, d_out, out_engine)
    if key not in _program_cache:
        _program_cache[key] = _build_program(m_pad, d_in, d_out, out_engine)
    nc = _program_cache[key]

    bf16 = ml_dtypes.bfloat16
    # weight[e].T, bf16 C-contiguous, built once per expert actually used
    wT_cache = {}
    for round0 in range(0, len(chunks), _N_CORES):
        batch = chunks[round0:round0 + _N_CORES]
        in_maps = []
        for e, r0, r1 in batch:
            xT = np.zeros((d_in, m_pad), dtype=bf16)
            if r1 > r0:
                xT[:, : r1 - r0] = x[r0:r1].T.astype(bf16)
            if e not in wT_cache:
                wT_cache[e] = np.ascontiguousarray(weight[e].T.astype(bf16))
            in_maps.append({"xT": xT, "wT": wT_cache[e]})

        res = run_bass_kernel_spmd(nc, in_maps, core_ids=list(range(_N_CORES)))

        for (e, r0, r1), out in zip(batch, res.results):
            if r1 > r0:
                y[r0:r1] = out["yT"][:, : r1 - r0].T.astype(np.float32)
    return y


# revision 9
# speedup vs baseline: 1.2364x; 1.1097x over previous
"""ColumnParallelFusedMoeLinear grouped-GEMM kernel for 8 Trainium2 NeuronCores.

Strategy (expert/token parallel):
  Tokens are sorted by expert; m_sizes gives each expert's contiguous row
  range of x.  The host splits the full token range into single-expert
  chunks (balanced m_sizes -> one expert per core) and each core computes
  y_chunk = x_chunk @ weight[e].T, with the host scattering chunk rows back
  into the full output.

  Inputs are cast to bf16 on the host (PE streams 1 column/cycle for bf16
  same as fp32r, but HBM traffic halves; K=1024 accumulation stays fp32 in
  PSUM so the result error is ~5e-3, well inside the 2e-2 gate).  The
  output is stored transposed (yT, bf16) and the host casts/transposes it
  back, halving the store traffic too.  Per-core HBM traffic ~10 MB vs a
  ~57 us matmul roofline, so the kernel is tensor-engine-bound.

  The matmul puts the weight column block [k=128, n=128] stationary and
  streams the token dim as the moving free dim, so the ragged per-expert
  token count (973..1065 here) is NOT padded up to a multiple of 128 --
  only to the SPMD-uniform m_pad.  PSUM limits the moving dim to 512 fp32
  per bank, so the token range is cut into ceil(m_pad/512) near-equal
  chunks (all >=256 wide, keeping LDWEIGHTS hidden under the stream).
"""

import math
import os

import ml_dtypes
import numpy as np

_N_CORES = 8
_P = 128
_PSUM_F = 512  # PSUM bank width in fp32 = max moving free dim per matmul

_program_cache = {}


def _m_chunks(m_pad):
    """Cut [0, m_pad) into near-equal chunks of <=512, multiples of 8."""
    n = (m_pad + _PSUM_F - 1) // _PSUM_F
    base = min(_PSUM_F, ((-(-m_pad // n) + 7) // 8) * 8)
    bounds = []
    c = 0
    while c < m_pad:
        bounds.append((c, min(c + base, m_pad)))
        c = bounds[-1][1]
    return bounds


def _build_program(m_pad, d_in, d_out, out_engine="scalar", warm_mms=8):
    import concourse.mybir as mybir
    import concourse.tile as tile
    from concourse import bacc

    kc_n = d_in // _P   # contraction chunks of 128
    nt_n = d_out // _P  # stationary weight column tiles
    chunks = _m_chunks(m_pad)

    nc = bacc.Bacc("TRN2", target_bir_lowering=False, debug=False)
    xT = nc.dram_tensor("xT", [d_in, m_pad], mybir.dt.bfloat16, kind="ExternalInput")
    wT = nc.dram_tensor("wT", [d_in, d_out], mybir.dt.bfloat16, kind="ExternalInput")
    yT = nc.dram_tensor("yT", [d_out, m_pad], mybir.dt.bfloat16, kind="ExternalOutput")

    xT3 = xT.rearrange("(kc p) m -> kc p m", p=_P)
    wT3 = wT.rearrange("(kc p) o -> kc p o", p=_P)
    y3 = yT.rearrange("(nt p) m -> nt p m", p=_P)

    with tile.TileContext(nc) as tc:
        with (
            tc.tile_pool(name="xw", bufs=1) as xwpool,
            tc.tile_pool(name="out", bufs=6) as outpool,
            tc.tile_pool(name="psum", bufs=6, space="PSUM") as psumpool,
            tc.tile_pool(name="warm", bufs=1, space="PSUM") as warmpool,
        ):
            # x and w live in single fused tiles (kc along the free dim) so
            # one DMA covers all 8 contraction chunks -- per-DMA fixed cost
            # (~0.6 us HWDGE) made many small DMAs throttle the ramp.
            xall = xwpool.tile([_P, kc_n * m_pad], mybir.dt.bfloat16, tag="xall",
                               name="xall")
            wall = xwpool.tile([_P, kc_n * d_out], mybir.dt.bfloat16, tag="wall",
                               name="wall")
            # partition dim stays outermost in the DMA APs (the race
            # detector / DGE model needs partition-major access patterns)
            xv = xall[:].rearrange("p (kc m) -> p kc m", kc=kc_n)
            wv = wall[:].rearrange("p (kc o) -> p kc o", kc=kc_n)
            xs = xT3.rearrange("kc p m -> p kc m")
            ws = wT3.rearrange("kc p o -> p kc o")

            # PE warm-up: ~3.4us of dummy matmuls with no DMA dependency so
            # the HAM clock gate reaches 8/8 while the input ramp streams in.
            if warm_mms:
                scr = xwpool.tile([_P, 640], mybir.dt.bfloat16, tag="scr",
                                  name="scr")
                nc.gpsimd.memset(scr[:], 0)
                psw = warmpool.tile([_P, _PSUM_F], mybir.dt.float32, tag="psw",
                                    name="psw")
                for i in range(warm_mms):
                    nc.tensor.matmul(psw[:], scr[:, 0:_P], scr[:, _P:640],
                                     start=(i == 0), stop=(i == warm_mms - 1))

            # DMA emission order == arrival order on the sync queue.  Cascade
            # small->large in consumption order: first two weight column
            # tiles, first x chunk (PE dense from ~3.5us), x remainder, then
            # weight column groups of doubling width.
            c0, c1 = chunks[0]
            nc.sync.dma_start(wv[:, :, 0:2 * _P], ws[:, :, 0:2 * _P])
            nc.sync.dma_start(xv[:, :, c0:c1], xs[:, :, c0:c1])
            if c1 < m_pad:
                nc.sync.dma_start(xv[:, :, c1:m_pad], xs[:, :, c1:m_pad])
            g0 = 2
            while g0 < nt_n:
                g1 = min(2 * g0, nt_n)
                nc.sync.dma_start(wv[:, :, g0 * _P:g1 * _P],
                                  ws[:, :, g0 * _P:g1 * _P])
                g0 = g1

            engines = {
                "gpsimd": nc.gpsimd.dma_start,
                "scalar": nc.scalar.dma_start,
                "sync": nc.sync.dma_start,
            }
            if out_engine == "alt":
                out_dmas = [engines["scalar"], engines["sync"]]
            else:
                out_dmas = [engines[out_engine]]

            for nt in range(nt_n):
                ps = [psumpool.tile([_P, _PSUM_F], mybir.dt.float32, tag="ps",
                                    name=f"ps{nt}_{ci}")
                      for ci in range(len(chunks))]
                o = outpool.tile([_P, m_pad], mybir.dt.bfloat16, tag="o")
                for ci, (c0, c1) in enumerate(chunks):
                    for kc in range(kc_n):
                        nc.tensor.matmul(
                            ps[ci][:, :c1 - c0],
                            wall[:, kc * d_out + nt * _P:
                                 kc * d_out + (nt + 1) * _P],
                            xall[:, kc * m_pad + c0:kc * m_pad + c1],
                            start=(kc == 0),
                            stop=(kc == kc_n - 1),
                        )
                    nc.vector.tensor_copy(o[:, c0:c1], ps[ci][:, :c1 - c0])
                out_dmas[nt % len(out_dmas)](y3[nt], o[:])
    nc.compile()
    return nc


# Largest chunk one core handles per SPMD round (SBUF-bounded).
_MAX_CHUNK = 2560


def _plan_chunks(m_sizes, T):
    """Split [0, T) into single-expert chunks, balanced by length.

    Every chunk is <= _MAX_CHUNK rows.  Returns a list of (expert, row0,
    row1) padded with empty (0, 0, 0) chunks to a multiple of _N_CORES,
    or None if there are no rows at all.
    """
    off = np.cumsum(np.asarray(m_sizes, dtype=np.int64))
    starts = np.clip(np.concatenate([[0], off[:-1]]), 0, T)
    ends = np.clip(off, 0, T)
    segs = [(e, int(starts[e]), int(ends[e]))
            for e in range(len(m_sizes)) if ends[e] > starts[e]]
    if not segs:
        return None
    lens = np.array([s1 - s0 for _, s0, s1 in segs], dtype=np.float64)
    # mandatory splits so no chunk exceeds _MAX_CHUNK, then distribute any
    # spare cores (up to the next multiple of _N_CORES) to the biggest shares
    n_chunks = np.ceil(lens / _MAX_CHUNK).astype(np.int64)
    total = int(n_chunks.sum())
    spare = (-total) % _N_CORES if total > _N_CORES else _N_CORES - total
    for _ in range(spare):
        i = int(np.argmax(lens / n_chunks))
        n_chunks[i] += 1
    chunks = []
    for (e, s0, s1), k in zip(segs, n_chunks):
        L = s1 - s0
        bounds = [s0 + (L * i) // k for i in range(int(k) + 1)]
        for i in range(int(k)):
            if bounds[i + 1] > bounds[i]:
                chunks.append((e, bounds[i], bounds[i + 1]))
    while len(chunks) % _N_CORES:
        chunks.append((0, 0, 0))
    return chunks


def kernel(x, weight, m_sizes):
    from concourse.bass_utils import run_bass_kernel_spmd

    x = np.ascontiguousarray(np.asarray(x), dtype=np.float32)
    weight = np.asarray(weight, dtype=np.float32)
    m_arr = np.asarray(m_sizes)

    T, d_in = x.shape
    E, d_out, _ = weight.shape

    y = np.zeros((T, d_out), dtype=np.float32)
    chunks = _plan_chunks(m_arr, T)
    if chunks is None:
        return y

    max_len = max(r1 - r0 for _, r0, r1 in chunks)
    m_pad = max(_P, int(math.ceil(max_len / 16)) * 16)

    out_engine = os.environ.get("MOE_OUT_ENGINE", "alt")
    warm_mms = int(os.environ.get("MOE_WARM_MMS", "8"))
    key = (m_pad, d_in, d_out, out_engine, warm_mms)
    if key not in _program_cache:
        _program_cache[key] = _build_program(m_pad, d_in, d_out, out_engine,
                                             warm_mms)
    nc = _program_cache[key]

    bf16 = ml_dtypes.bfloat16
    # weight[e].T, bf16 C-contiguous, built once per expert actually used
    wT_cache = {}
    for round0 in range(0, len(chunks), _N_CORES):
        batch = chunks[round0:round0 + _N_CORES]
        in_maps = []
        for e, r0, r1 in batch:
            xT = np.zeros((d_in, m_pad), dtype=bf16)
            if r1 > r0:
                xT[:, : r1 - r0] = x[r0:r1].T.astype(bf16)
            if e not in wT_cache:
                wT_cache[e] = np.ascontiguousarray(weight[e].T.astype(bf16))
            in_maps.append({"xT": xT, "wT": wT_cache[e]})

        res = run_bass_kernel_spmd(nc, in_maps, core_ids=list(range(_N_CORES)))

        for (e, r0, r1), out in zip(batch, res.results):
            if r1 > r0:
                y[r0:r1] = out["yT"][:, : r1 - r0].T.astype(np.float32)
    return y


# revision 11
# speedup vs baseline: 1.2520x; 1.0126x over previous
"""ColumnParallelFusedMoeLinear grouped-GEMM kernel for 8 Trainium2 NeuronCores.

Strategy (expert/token parallel):
  Tokens are sorted by expert; m_sizes gives each expert's contiguous row
  range of x.  The host splits the full token range into single-expert
  chunks (balanced m_sizes -> one expert per core) and each core computes
  y_chunk = x_chunk @ weight[e].T, with the host scattering chunk rows back
  into the full output.

  Inputs are cast to bf16 on the host (PE streams 1 column/cycle for bf16
  same as fp32r, but HBM traffic halves; K=1024 accumulation stays fp32 in
  PSUM so the result error is ~5e-3, well inside the 2e-2 gate).  The
  output is stored transposed (yT, bf16) and the host casts/transposes it
  back, halving the store traffic too.  Per-core HBM traffic ~10 MB vs a
  ~57 us matmul roofline, so the kernel is tensor-engine-bound.

  The matmul puts the weight column block [k=128, n=128] stationary and
  streams the token dim as the moving free dim, so the ragged per-expert
  token count (973..1065 here) is NOT padded up to a multiple of 128 --
  only to the SPMD-uniform m_pad.  PSUM limits the moving dim to 512 fp32
  per bank, so the token range is cut into ceil(m_pad/512) near-equal
  chunks (all >=256 wide, keeping LDWEIGHTS hidden under the stream).
"""

import math
import os

import ml_dtypes
import numpy as np

_N_CORES = 8
_P = 128
_PSUM_F = 512  # PSUM bank width in fp32 = max moving free dim per matmul

_program_cache = {}


def _m_chunks(m_pad):
    """Cut [0, m_pad) into near-equal chunks of <=512, multiples of 8."""
    n = (m_pad + _PSUM_F - 1) // _PSUM_F
    base = min(_PSUM_F, ((-(-m_pad // n) + 7) // 8) * 8)
    bounds = []
    c = 0
    while c < m_pad:
        bounds.append((c, min(c + base, m_pad)))
        c = bounds[-1][1]
    return bounds


def _w_groups(nt_n):
    """Weight column-tile groups in consumption order: [0:2] then doubling."""
    groups = [(0, min(2, nt_n))]
    while groups[-1][1] < nt_n:
        g0 = groups[-1][1]
        groups.append((g0, min(2 * g0, nt_n)))
    return groups


def _build_program(m_pad, d_in, d_out, out_engine="scalar", warm_mms=10):
    import concourse.mybir as mybir
    import concourse.tile as tile
    from concourse import bacc

    kc_n = d_in // _P   # contraction chunks of 128
    nt_n = d_out // _P  # stationary weight column tiles
    chunks = _m_chunks(m_pad)
    groups = _w_groups(nt_n)

    # Hosts packs x as [p, ci, kc, m_chunk] and w as [p, g, kc, cols] so
    # every DMA is a plain 2D contiguous slice with multi-KB partition
    # lines (small-line ramp DMAs ran at ~150 GB/s, these hit line rate).
    x_off = {}
    off = 0
    for ci, (c0, c1) in enumerate(chunks):
        x_off[ci] = off
        off += kc_n * (c1 - c0)
    w_off = {}
    off = 0
    for gi, (g0, g1) in enumerate(groups):
        w_off[gi] = off
        off += kc_n * (g1 - g0) * _P

    nc = bacc.Bacc("TRN2", target_bir_lowering=False, debug=False)
    xT = nc.dram_tensor("xT", [_P, kc_n * m_pad], mybir.dt.bfloat16,
                        kind="ExternalInput")
    wT = nc.dram_tensor("wT", [_P, kc_n * d_out], mybir.dt.bfloat16,
                        kind="ExternalInput")
    yT = nc.dram_tensor("yT", [d_out, m_pad], mybir.dt.bfloat16,
                        kind="ExternalOutput")
    y3 = yT.rearrange("(nt p) m -> nt p m", p=_P)

    with tile.TileContext(nc) as tc:
        with (
            tc.tile_pool(name="xw", bufs=1) as xwpool,
            tc.tile_pool(name="out", bufs=16) as outpool,
            tc.tile_pool(name="psum", bufs=6, space="PSUM") as psumpool,
            tc.tile_pool(name="warm", bufs=1, space="PSUM") as warmpool,
        ):
            xall = xwpool.tile([_P, kc_n * m_pad], mybir.dt.bfloat16,
                               tag="xall", name="xall")
            wall = xwpool.tile([_P, kc_n * d_out], mybir.dt.bfloat16,
                               tag="wall", name="wall")

            # PE warm-up: dummy matmuls with no DMA dependency so the HAM
            # clock gate reaches 8/8 while the input ramp streams in.  DVE
            # memset (gpsimd's Q7 startup is multi-us).
            if warm_mms:
                scr = xwpool.tile([_P, 640], mybir.dt.bfloat16, tag="scr",
                                  name="scr")
                nc.vector.memset(scr[:], 0)
                psw = warmpool.tile([_P, _PSUM_F], mybir.dt.float32, tag="psw",
                                    name="psw")
                for i in range(warm_mms):
                    nc.tensor.matmul(psw[:], scr[:, 0:_P], scr[:, _P:640],
                                     start=(i == 0), stop=(i == warm_mms - 1))

            # DMA emission order == arrival order on the sync queue, in
            # consumption order: w group0, x chunk0, x rest, w groups 1..
            c0, c1 = chunks[0]
            nc.sync.dma_start(wall[:, 0:w_off[1]], wT[:, 0:w_off[1]])
            nc.sync.dma_start(xall[:, 0:x_off.get(1, kc_n * m_pad)],
                              xT[:, 0:x_off.get(1, kc_n * m_pad)])
            if len(chunks) > 1:
                nc.sync.dma_start(xall[:, x_off[1]:kc_n * m_pad],
                                  xT[:, x_off[1]:kc_n * m_pad])
            for gi in range(1, len(groups)):
                e = w_off[gi + 1] if gi + 1 < len(groups) else kc_n * d_out
                nc.sync.dma_start(wall[:, w_off[gi]:e], wT[:, w_off[gi]:e])

            out_dma = {
                "gpsimd": nc.gpsimd.dma_start,
                "scalar": nc.scalar.dma_start,
                "sync": nc.sync.dma_start,
            }[out_engine]

            nt_group = {}
            for gi, (g0, g1) in enumerate(groups):
                for nt in range(g0, g1):
                    nt_group[nt] = (gi, g0, g1)

            for nt in range(nt_n):
                gi, g0, g1 = nt_group[nt]
                ps = [psumpool.tile([_P, _PSUM_F], mybir.dt.float32, tag="ps",
                                    name=f"ps{nt}_{ci}")
                      for ci in range(len(chunks))]
                o = outpool.tile([_P, m_pad], mybir.dt.bfloat16, tag="o")
                for ci, (c0, c1) in enumerate(chunks):
                    for kc in range(kc_n):
                        woff = (w_off[gi] + kc * (g1 - g0) * _P
                                + (nt - g0) * _P)
                        nc.tensor.matmul(
                            ps[ci][:, :c1 - c0],
                            wall[:, woff:woff + _P],
                            xall[:, x_off[ci] + kc * (c1 - c0):
                                 x_off[ci] + (kc + 1) * (c1 - c0)],
                            start=(kc == 0),
                            stop=(kc == kc_n - 1),
                        )
                    nc.vector.tensor_copy(o[:, c0:c1], ps[ci][:, :c1 - c0])
                out_dma(y3[nt], o[:])
    nc.compile()
    return nc


def _pack_x(xT_b, chunks, kc_n):
    """[d_in, m_pad] bf16 -> [128, sum_ci kc_n*len_ci] in [ci][kc][m] order."""
    v = xT_b.reshape(kc_n, _P, -1).transpose(1, 0, 2)  # [p, kc, m]
    return np.concatenate(
        [np.ascontiguousarray(v[:, :, c0:c1]).reshape(_P, -1)
         for c0, c1 in chunks], axis=1)


def _pack_w(wT_b, groups, kc_n):
    """[d_in, d_out] bf16 -> [128, kc_n*d_out] in [g][kc][cols] order."""
    v = wT_b.reshape(kc_n, _P, -1).transpose(1, 0, 2)  # [p, kc, o]
    return np.concatenate(
        [np.ascontiguousarray(v[:, :, g0 * _P:g1 * _P]).reshape(_P, -1)
         for g0, g1 in groups], axis=1)


# Largest chunk one core handles per SPMD round (SBUF-bounded).
_MAX_CHUNK = 2560


def _plan_chunks(m_sizes, T):
    """Split [0, T) into single-expert chunks, balanced by length.

    Every chunk is <= _MAX_CHUNK rows.  Returns a list of (expert, row0,
    row1) padded with empty (0, 0, 0) chunks to a multiple of _N_CORES,
    or None if there are no rows at all.
    """
    off = np.cumsum(np.asarray(m_sizes, dtype=np.int64))
    starts = np.clip(np.concatenate([[0], off[:-1]]), 0, T)
    ends = np.clip(off, 0, T)
    segs = [(e, int(starts[e]), int(ends[e]))
            for e in range(len(m_sizes)) if ends[e] > starts[e]]
    if not segs:
        return None
    lens = np.array([s1 - s0 for _, s0, s1 in segs], dtype=np.float64)
    # mandatory splits so no chunk exceeds _MAX_CHUNK, then distribute any
    # spare cores (up to the next multiple of _N_CORES) to the biggest shares
    n_chunks = np.ceil(lens / _MAX_CHUNK).astype(np.int64)
    total = int(n_chunks.sum())
    spare = (-total) % _N_CORES if total > _N_CORES else _N_CORES - total
    for _ in range(spare):
        i = int(np.argmax(lens / n_chunks))
        n_chunks[i] += 1
    chunks = []
    for (e, s0, s1), k in zip(segs, n_chunks):
        L = s1 - s0
        bounds = [s0 + (L * i) // k for i in range(int(k) + 1)]
        for i in range(int(k)):
            if bounds[i + 1] > bounds[i]:
                chunks.append((e, bounds[i], bounds[i + 1]))
    while len(chunks) % _N_CORES:
        chunks.append((0, 0, 0))
    return chunks


def kernel(x, weight, m_sizes):
    from concourse.bass_utils import run_bass_kernel_spmd

    x = np.ascontiguousarray(np.asarray(x), dtype=np.float32)
    weight = np.asarray(weight, dtype=np.float32)
    m_arr = np.asarray(m_sizes)

    T, d_in = x.shape
    E, d_out, _ = weight.shape

    y = np.zeros((T, d_out), dtype=np.float32)
    chunks = _plan_chunks(m_arr, T)
    if chunks is None:
        return y

    max_len = max(r1 - r0 for _, r0, r1 in chunks)
    m_pad = max(_P, int(math.ceil(max_len / 16)) * 16)

    out_engine = os.environ.get("MOE_OUT_ENGINE", "scalar")
    warm_mms = int(os.environ.get("MOE_WARM_MMS", "10"))
    key = (m_pad, d_in, d_out, out_engine, warm_mms)
    if key not in _program_cache:
        _program_cache[key] = _build_program(m_pad, d_in, d_out, out_engine,
                                             warm_mms)
    nc = _program_cache[key]

    bf16 = ml_dtypes.bfloat16
    kc_n = d_in // _P
    m_chunks = _m_chunks(m_pad)
    groups = _w_groups(d_out // _P)
    # weight[e].T packed, built once per expert actually used
    wT_cache = {}
    for round0 in range(0, len(chunks), _N_CORES):
        batch = chunks[round0:round0 + _N_CORES]
        in_maps = []
        for e, r0, r1 in batch:
            xT = np.zeros((d_in, m_pad), dtype=bf16)
            if r1 > r0:
                xT[:, : r1 - r0] = x[r0:r1].T.astype(bf16)
            if e not in wT_cache:
                wT_cache[e] = _pack_w(weight[e].T.astype(bf16), groups, kc_n)
            in_maps.append({"xT": _pack_x(xT, m_chunks, kc_n),
                            "wT": wT_cache[e]})

        res = run_bass_kernel_spmd(nc, in_maps, core_ids=list(range(_N_CORES)))

        for (e, r0, r1), out in zip(batch, res.results):
            if r1 > r0:
                y[r0:r1] = out["yT"][:, : r1 - r0].T.astype(np.float32)
    return y


# revision 16
# speedup vs baseline: 1.2818x; 1.0238x over previous
"""ColumnParallelFusedMoeLinear grouped-GEMM kernel for 8 Trainium2 NeuronCores.

Strategy (expert/token parallel):
  Tokens are sorted by expert; m_sizes gives each expert's contiguous row
  range of x.  The host splits the full token range into single-expert
  chunks (balanced m_sizes -> one expert per core) and each core computes
  y_chunk = x_chunk @ weight[e].T, with the host scattering chunk rows back
  into the full output.

  Inputs are cast to bf16 on the host (PE streams 1 column/cycle for bf16
  same as fp32r, but HBM traffic halves; K=1024 accumulation stays fp32 in
  PSUM so the result error is ~5e-3, well inside the 2e-2 gate).  The
  output is stored transposed (yT, bf16) and the host casts/transposes it
  back, halving the store traffic too.  Per-core HBM traffic ~10 MB vs a
  ~57 us matmul roofline, so the kernel is tensor-engine-bound.

  The matmul puts the weight column block [k=128, n=128] stationary and
  streams the token dim as the moving free dim, so the ragged per-expert
  token count (973..1065 here) is NOT padded up to a multiple of 128 --
  only to the SPMD-uniform m_pad.  PSUM limits the moving dim to 512 fp32
  per bank, so the token range is cut into ceil(m_pad/512) near-equal
  chunks (all >=256 wide, keeping LDWEIGHTS hidden under the stream).
"""

import math
import os

import ml_dtypes
import numpy as np

_N_CORES = 8
_P = 128
_PSUM_F = 512  # PSUM bank width in fp32 = max moving free dim per matmul

_program_cache = {}


def _m_chunks(m_pad):
    """Cut [0, m_pad) into chunks of <=512 (multiples of 8).

    The first chunk is small (256) so the first x DMA lands early and the
    PE can start the first accumulation group sooner; the rest near-equal.
    """
    if m_pad <= _PSUM_F:
        return [(0, m_pad)]
    bounds = [(0, 256)]
    rest = m_pad - 256
    n = (rest + _PSUM_F - 1) // _PSUM_F
    base = min(_PSUM_F, ((-(-rest // n) + 7) // 8) * 8)
    c = 256
    while c < m_pad:
        bounds.append((c, min(c + base, m_pad)))
        c = bounds[-1][1]
    return bounds


def _w_groups(nt_n):
    """Weight column-tile groups in consumption order: two singles, then
    doubling widths -- small DMAs early (low latency), big ones late."""
    groups = [(0, 1), (1, 2)] if nt_n > 1 else [(0, 1)]
    while groups[-1][1] < nt_n:
        g0 = groups[-1][1]
        groups.append((g0, min(2 * g0, nt_n)))
    return groups


def _build_program(m_pad, d_in, d_out, out_engine="scalar", warm_mms=10):
    import concourse.mybir as mybir
    import concourse.tile as tile
    from concourse import bacc

    kc_n = d_in // _P   # contraction chunks of 128
    nt_n = d_out // _P  # stationary weight column tiles
    chunks = _m_chunks(m_pad)
    groups = _w_groups(nt_n)

    # Hosts packs x as [p, ci, kc, m_chunk] and w as [p, g, kc, cols] so
    # every DMA is a plain 2D contiguous slice with multi-KB partition
    # lines (small-line ramp DMAs ran at ~150 GB/s, these hit line rate).
    x_off = {}
    off = 0
    for ci, (c0, c1) in enumerate(chunks):
        x_off[ci] = off
        off += kc_n * (c1 - c0)
    w_off = {}
    off = 0
    for gi, (g0, g1) in enumerate(groups):
        w_off[gi] = off
        off += kc_n * (g1 - g0) * _P

    nc = bacc.Bacc("TRN2", target_bir_lowering=False, debug=False)
    xT = nc.dram_tensor("xT", [_P, kc_n * m_pad], mybir.dt.bfloat16,
                        kind="ExternalInput")
    wT = nc.dram_tensor("wT", [_P, kc_n * d_out], mybir.dt.bfloat16,
                        kind="ExternalInput")
    yT = nc.dram_tensor("yT", [d_out, m_pad], mybir.dt.bfloat16,
                        kind="ExternalOutput")
    y3 = yT.rearrange("(nt p) m -> nt p m", p=_P)

    with tile.TileContext(nc) as tc:
        with (
            tc.tile_pool(name="xw", bufs=1) as xwpool,
            tc.tile_pool(name="out", bufs=16) as outpool,
            tc.tile_pool(name="psum", bufs=6, space="PSUM") as psumpool,
            tc.tile_pool(name="warm", bufs=1, space="PSUM") as warmpool,
        ):
            xall = xwpool.tile([_P, kc_n * m_pad], mybir.dt.bfloat16,
                               tag="xall", name="xall")
            wall = xwpool.tile([_P, kc_n * d_out], mybir.dt.bfloat16,
                               tag="wall", name="wall")

            # PE warm-up: dummy matmuls with no DMA dependency so the HAM
            # clock gate reaches 8/8 while the input ramp streams in.  DVE
            # memset (gpsimd's Q7 startup is multi-us).
            if warm_mms:
                scr = xwpool.tile([_P, 640], mybir.dt.bfloat16, tag="scr",
                                  name="scr")
                nc.vector.memset(scr[:], 0)
                psw = warmpool.tile([_P, _PSUM_F], mybir.dt.float32, tag="psw",
                                    name="psw")
                for i in range(warm_mms):
                    nc.tensor.matmul(psw[:], scr[:, 0:_P], scr[:, _P:640],
                                     start=(i == 0), stop=(i == warm_mms - 1))

            # DMA emission order == arrival order on the sync queue, in
            # consumption order with small transfers first (per-DMA
            # completion receipt ~1.5us sits on every dependency edge):
            # w n0, x c0, x c1, w n1, x c2.., then w groups of doubling size.
            def wslice(gi):
                e = w_off[gi + 1] if gi + 1 < len(groups) else kc_n * d_out
                return w_off[gi], e

            def xslice(ci):
                e = x_off[ci + 1] if ci + 1 < len(chunks) else kc_n * m_pad
                return x_off[ci], e

            order = [("w", 0), ("x", 0)]
            if len(chunks) > 1:
                order.append(("x", 1))
            if len(groups) > 1:
                order.append(("w", 1))
            order += [("x", ci) for ci in range(2, len(chunks))]
            order += [("w", gi) for gi in range(2, len(groups))]
            for kind, i in order:
                s, e = wslice(i) if kind == "w" else xslice(i)
                src, dst = (wT, wall) if kind == "w" else (xT, xall)
                nc.sync.dma_start(dst[:, s:e], src[:, s:e])

            out_dma = {
                "gpsimd": nc.gpsimd.dma_start,
                "scalar": nc.scalar.dma_start,
                "sync": nc.sync.dma_start,
            }[out_engine]

            nt_group = {}
            for gi, (g0, g1) in enumerate(groups):
                for nt in range(g0, g1):
                    nt_group[nt] = (gi, g0, g1)

            for nt in range(nt_n):
                gi, g0, g1 = nt_group[nt]
                ps = [psumpool.tile([_P, _PSUM_F], mybir.dt.float32, tag="ps",
                                    name=f"ps{nt}_{ci}")
                      for ci in range(len(chunks))]
                o = outpool.tile([_P, m_pad], mybir.dt.bfloat16, tag="o")
                for ci, (c0, c1) in enumerate(chunks):
                    for kc in range(kc_n):
                        woff = (w_off[gi] + kc * (g1 - g0) * _P
                                + (nt - g0) * _P)
                        nc.tensor.matmul(
                            ps[ci][:, :c1 - c0],
                            wall[:, woff:woff + _P],
                            xall[:, x_off[ci] + kc * (c1 - c0):
                                 x_off[ci] + (kc + 1) * (c1 - c0)],
                            start=(kc == 0),
                            stop=(kc == kc_n - 1),
                        )
                    nc.vector.tensor_copy(o[:, c0:c1], ps[ci][:, :c1 - c0])
                    if nt >= nt_n - 2:
                        # per-chunk stores at the end shorten the drain tail
                        out_dma(y3[nt, :, c0:c1], o[:, c0:c1])
                if nt < nt_n - 2:
                    out_dma(y3[nt], o[:])
    nc.compile()
    return nc


def _pack_x(xT_b, chunks, kc_n):
    """[d_in, m_pad] bf16 -> [128, sum_ci kc_n*len_ci] in [ci][kc][m] order."""
    v = xT_b.reshape(kc_n, _P, -1).transpose(1, 0, 2)  # [p, kc, m]
    return np.concatenate(
        [np.ascontiguousarray(v[:, :, c0:c1]).reshape(_P, -1)
         for c0, c1 in chunks], axis=1)


def _pack_w(wT_b, groups, kc_n):
    """[d_in, d_out] bf16 -> [128, kc_n*d_out] in [g][kc][cols] order."""
    v = wT_b.reshape(kc_n, _P, -1).transpose(1, 0, 2)  # [p, kc, o]
    return np.concatenate(
        [np.ascontiguousarray(v[:, :, g0 * _P:g1 * _P]).reshape(_P, -1)
         for g0, g1 in groups], axis=1)


# Largest chunk one core handles per SPMD round (SBUF-bounded).
_MAX_CHUNK = 2560


def _plan_chunks(m_sizes, T):
    """Split [0, T) into single-expert chunks, balanced by length.

    Every chunk is <= _MAX_CHUNK rows.  Returns a list of (expert, row0,
    row1) padded with empty (0, 0, 0) chunks to a multiple of _N_CORES,
    or None if there are no rows at all.
    """
    off = np.cumsum(np.asarray(m_sizes, dtype=np.int64))
    starts = np.clip(np.concatenate([[0], off[:-1]]), 0, T)
    ends = np.clip(off, 0, T)
    segs = [(e, int(starts[e]), int(ends[e]))
            for e in range(len(m_sizes)) if ends[e] > starts[e]]
    if not segs:
        return None
    lens = np.array([s1 - s0 for _, s0, s1 in segs], dtype=np.float64)
    # mandatory splits so no chunk exceeds _MAX_CHUNK, then distribute any
    # spare cores (up to the next multiple of _N_CORES) to the biggest shares
    n_chunks = np.ceil(lens / _MAX_CHUNK).astype(np.int64)
    total = int(n_chunks.sum())
    spare = (-total) % _N_CORES if total > _N_CORES else _N_CORES - total
    for _ in range(spare):
        i = int(np.argmax(lens / n_chunks))
        n_chunks[i] += 1
    chunks = []
    for (e, s0, s1), k in zip(segs, n_chunks):
        L = s1 - s0
        bounds = [s0 + (L * i) // k for i in range(int(k) + 1)]
        for i in range(int(k)):
            if bounds[i + 1] > bounds[i]:
                chunks.append((e, bounds[i], bounds[i + 1]))
    while len(chunks) % _N_CORES:
        chunks.append((0, 0, 0))
    return chunks


def kernel(x, weight, m_sizes):
    from concourse.bass_utils import run_bass_kernel_spmd

    x = np.ascontiguousarray(np.asarray(x), dtype=np.float32)
    weight = np.asarray(weight, dtype=np.float32)
    m_arr = np.asarray(m_sizes)

    T, d_in = x.shape
    E, d_out, _ = weight.shape

    y = np.zeros((T, d_out), dtype=np.float32)
    chunks = _plan_chunks(m_arr, T)
    if chunks is None:
        return y

    max_len = max(r1 - r0 for _, r0, r1 in chunks)
    m_pad = max(_P, int(math.ceil(max_len / 16)) * 16)

    out_engine = os.environ.get("MOE_OUT_ENGINE", "scalar")
    warm_mms = int(os.environ.get("MOE_WARM_MMS", "12"))
    key = (m_pad, d_in, d_out, out_engine, warm_mms)
    if key not in _program_cache:
        _program_cache[key] = _build_program(m_pad, d_in, d_out, out_engine,
                                             warm_mms)
    nc = _program_cache[key]

    bf16 = ml_dtypes.bfloat16
    kc_n = d_in // _P
    m_chunks = _m_chunks(m_pad)
    groups = _w_groups(d_out // _P)
    # weight[e].T packed, built once per expert actually used
    wT_cache = {}
    for round0 in range(0, len(chunks), _N_CORES):
        batch = chunks[round0:round0 + _N_CORES]
        in_maps = []
        for e, r0, r1 in batch:
            xT = np.zeros((d_in, m_pad), dtype=bf16)
            if r1 > r0:
                xT[:, : r1 - r0] = x[r0:r1].T.astype(bf16)
            if e not in wT_cache:
                wT_cache[e] = _pack_w(weight[e].T.astype(bf16), groups, kc_n)
            in_maps.append({"xT": _pack_x(xT, m_chunks, kc_n),
                            "wT": wT_cache[e]})

        res = run_bass_kernel_spmd(nc, in_maps, core_ids=list(range(_N_CORES)))

        for (e, r0, r1), out in zip(batch, res.results):
            if r1 > r0:
                y[r0:r1] = out["yT"][:, : r1 - r0].T.astype(np.float32)
    return y
